# revision 18
# baseline (speedup 1.0000x reference)
"""Linear self-attention (elu+1 feature map) Trainium2 kernel — bf16.

Problem: B=4, S=4096, D=1024, H=16, HD=64.
  q = elu1(x @ Wq.T + bq); k = elu1(x @ Wk.T + bk); v = x @ Wv.T + bv
  kv_h = k_h^T v_h; ksum_h = sum_t k_h; z = 1/(q.ksum + eps)
  out = (q_h @ kv_h) * z; y = out @ Wo.T + bo
Sharding: token-parallel. Core c handles batch c//2, sequence half c%2
(2048 tokens). kv/ksum are partial sums over local tokens, AllReduced
(bf16) across the 2-core group sharing a batch, then every core
finishes its own tokens through attention + output projection. bo is
added on host.

All PE-facing operands are bf16 (1 cycle/row on the PE; fp32 runs at
2). PSUM accumulation stays fp32. q^T stays resident in SBUF.

Schedule notes:
 - Weights are packed HALF-MAJOR (all chunks' first 512 output dims,
   then all second 512), so one 1MB DMA delivers a full projection
   half. DMA triggers cost ~0.5us of queue time each and queues starve
   for instruction fetch under HBM pressure, so loads are few and
   queue-serial in urgency order: wk (3 pieces) then wv/wq/wo (2 each)
   on gpsimd; x group 0 (4 pieces) then groups 1-3 (1 each, behind wv)
   on sync.
 - Warmup: the 8 group-0 k half-projections run before any
   v-projection, g0 halves first — they need only wk's first 1MB +
   x group 0, so the PE starts ~1us after the preamble and never waits
   for wv.
 - Pass 1 pipeline per tile: k-proj -> v-proj with the previous tile's
   8 kv matmuls interleaved between v half-projections; elu runs on
   DVE/ACT under the next tile's matmuls; the [v|1] assembly copy runs
   on ACT (the DVE is the straggler engine in pass 1).
 - The kv AllReduce launches right after the last kv matmul; the 4 q^T
   projection groups (~55us of matmuls) hide its latency. attn tiles
   0-1 are emitted between q^T groups 2 and 3 so their DVE normalize
   chains drain before the q^T group-3 elu backlog, removing the
   pass-2 pipeline-fill stalls.
 - Pass 2 per tile: 8 attn matmuls with 130-col rhs (the useful
   [qkv | den] block), DVE normalizes (bank 2 first — it is the only
   single-buffered od bank), the PE transposes the normalized output
   via is_transpose matmuls into a bf16 PSUM bank (a DMA transpose
   here costs 256KB/tile of 2-byte-gather SBUF traffic that halves
   concurrent matmul throughput), the DVE copies it back to SBUF, and
   the Wo matmuls consume it 3 tiles later, split into halves so y
   copies/stores pipeline with the drain.
 - PSUM budget: pass 1: kv 4 + proj 4; q^T phase: qh 3 + od 5;
   pass 2: od 5 + y halves 2 + transpose 1 = 8.
"""

import numpy as np
from contextlib import ExitStack

import concourse.bass as bass
import concourse.tile as tile
from concourse import bacc, mybir
from concourse.bass_utils import run_bass_kernel_spmd
from concourse.tile_rust import add_dep_helper

B, S, D, H, HD = 4, 4096, 1024, 16, 64
N_CORES = 8
TOK = (B * S) // N_CORES      # 2048 tokens per core
NT = TOK // 128               # 16 token tiles per core
GT = 4                        # token tiles per x^T group
NG = NT // GT
F32 = mybir.dt.float32
BF16 = mybir.dt.bfloat16
EPS = 1e-6

MM_DT = BF16

TRACE = False            # set by test harness for profiling
LAST_RESULT = None       # BassKernelResults of last run

_PROGRAMS = {}


def _emit(nc, has_bias, mm_dt):
    AF = mybir.ActivationFunctionType
    ALU = mybir.AluOpType

    # x^T, chunk-major per token tile within each group:
    # [p, g*4096 + c*512 + u] = x[g*512 + u, c*128 + p]
    xst = nc.dram_tensor("xst", [128, NT * 1024], mm_dt, kind="ExternalInput").ap()
    # weights half-major: [p, h*4096 + c*512 + n] = W.T[c*128+p, h*512+n]
    wkd = nc.dram_tensor("wkt", [128, 8 * D], mm_dt, kind="ExternalInput").ap()
    wvd = nc.dram_tensor("wvt", [128, 8 * D], mm_dt, kind="ExternalInput").ap()
    wqd = nc.dram_tensor("wqt", [128, 8 * D], mm_dt, kind="ExternalInput").ap()
    wod = nc.dram_tensor("wot", [128, 8 * D], mm_dt, kind="ExternalInput").ap()
    identd = nc.dram_tensor("ident", [128, 128], mm_dt, kind="ExternalInput").ap()
    biasd = nc.dram_tensor("biases", [1, 4096], F32, kind="ExternalInput").ap()
    # y ships bf16 (host upcasts); halves store traffic and the drain tail
    y_d = nc.dram_tensor("y", [TOK, D], mm_dt, kind="ExternalOutput").ap()
    # kv collective payload keeps the PSUM block layout: block r (heads
    # 2r, 2r+1) at cols r*130; rows 0:64 x 0:65 = [kv_2r | ksum_2r],
    # rows 64:128 x 65:130 = [kv_2r+1 | ksum_2r+1] (complement is junk)
    cc_in = nc.dram_tensor("cc_in", [128, 1040], mm_dt).ap()
    cc_out = nc.dram_tensor("cc_out", [128, 1040], mm_dt).ap()

    def wslice(wt, g, c, n0=0, nn=512):
        # half-major weight slice: out-half g, chunk c, cols n0:n0+nn
        return wt[:, g * 4096 + c * 512 + n0: g * 4096 + c * 512 + n0 + nn]

    with tile.TileContext(nc) as tc, ExitStack() as top:
        wpool = top.enter_context(tc.tile_pool(name="w", bufs=4))
        cpool = top.enter_context(tc.tile_pool(name="const", bufs=1))
        qtpool = top.enter_context(tc.tile_pool(name="qt", bufs=1))
        ospool = top.enter_context(tc.tile_pool(name="os", bufs=4))
        otpool = top.enter_context(tc.tile_pool(name="ot", bufs=5))
        zpool = top.enter_context(tc.tile_pool(name="z", bufs=2))
        # identity for the pass-2 PE transposes
        identm = cpool.tile([128, 128], mm_dt, tag="ident")
        nc.scalar.dma_start(identm[:], identd)
        # block-diagonal [kv | ksum] matrix for pass 2 (chunk c = heads
        # 2c, 2c+1); zeroed now while the DVE is idle, filled after the CC
        bd = cpool.tile([128, 2048], mm_dt, tag="bd")
        nc.vector.memset(bd[:].bitcast(F32), 0.0)
        if has_bias:
            ones_row_st = cpool.tile([1, 512], F32, tag="ones_row_st")
            nc.vector.memset(ones_row_st[:], 1.0)
            ones_row = cpool.tile([1, 128], mm_dt, tag="ones_row")
            nc.vector.tensor_copy(ones_row[:], ones_row_st[0:1, 0:128])
            ones_row512 = cpool.tile([1, 512], mm_dt, tag="ones_row512")
            nc.vector.tensor_copy(ones_row512[:], ones_row_st[:])
            bias_st = cpool.tile([1, 3072], F32, tag="bias_st")
            nc.sync.dma_start(bias_st[:], biasd[0:1, 0:3072])
            bias_sb = cpool.tile([1, 3072], mm_dt, tag="bias")
            nc.vector.tensor_copy(bias_sb[:], bias_st[:])

        # --- weight loads: few large DMAs, queue-serial on gpsimd in
        # urgency order; half-major layout means piece 0 serves all g=0
        # half-projections ---
        wk_t = wpool.tile([128, 8 * D], mm_dt, tag="w", name="wk")
        # finer pieces: warmup consumes wk chunk-by-chunk at ~450GB/s, so
        # lumpy arrival directly stalls the PE
        for lo, hi in ((0, 256), (256, 512), (512, 1024), (1024, 2048),
                       (2048, 3072), (3072, 4096), (4096, 6144), (6144, 8192)):
            nc.gpsimd.dma_start(wk_t[:, lo:hi], wkd[:, lo:hi])

        def load_weight_big(dram_ap, name):
            wt = wpool.tile([128, 8 * D], mm_dt, tag="w", name=name)
            last = None
            for hf in range(2):
                last = nc.gpsimd.dma_start(
                    wt[:, hf * 4096:(hf + 1) * 4096],
                    dram_ap[:, hf * 4096:(hf + 1) * 4096])
            return wt, last

        wv_t, wv_last = load_weight_big(wvd, "wv")
        wq_t, _ = load_weight_big(wqd, "wq")
        wo_t, _ = load_weight_big(wod, "wo")

        kvstack = ExitStack()
        kvpool = kvstack.enter_context(tc.tile_pool(name="kvp", bufs=1, space="PSUM"))
        # 2-head-batched kv accumulator: block r (heads 2r, 2r+1) at cols
        # r*256: rows 0:64 x cols 0:65 = [kv_2r | ksum_2r], rows 64:128 x
        # cols 65:130 = [kv_2r+1 | ksum_2r+1]; the other corners are unused
        kv_ps = kvpool.tile([128, 2048], F32, tag="kv")

        qts = {}
        osbs = {}
        NBLK = (3, 3, 2)   # attn blocks per PSUM bank (8 = 3+3+2)

        p1o = ExitStack()
        xtpool = p1o.enter_context(tc.tile_pool(name="xt", bufs=4))
        mepool = p1o.enter_context(tc.tile_pool(name="me", bufs=4))

        def elu1_half(dst_half, ps_half):
            # elu(x)+1 = exp(min(x,0)) + max(x,0), on a [128,512] half
            me = mepool.tile([128, 512], F32, tag="me")
            nc.vector.tensor_scalar_min(me[:], ps_half, 0.0)
            nc.scalar.activation(me[:], me[:], AF.Exp)
            nc.vector.scalar_tensor_tensor(
                dst_half, ps_half, 0.0, me[:], ALU.max, ALU.add)

        # x loads: group 0 in 4 pieces (first 128 cols split off so matmul
        # 0 starts sooner); groups 1-3 single 1MB DMAs, queue-serial on
        # sync, group 1 deferred behind wv so early HBM serves wk+x0+wv —
        # the bytes the PE actually waits for
        xtgs = {}
        for g in range(NG):
            xtgs[g] = xtpool.tile([128, GT * 1024], mm_dt, tag="xt",
                                  name=f"xtg{g}")
        nc.sync.dma_start(xtgs[0][:, 0:128], xst[:, 0:128])
        nc.sync.dma_start(xtgs[0][:, 128:1024], xst[:, 128:1024])
        nc.sync.dma_start(xtgs[0][:, 1024:2560], xst[:, 1024:2560])
        nc.sync.dma_start(xtgs[0][:, 2560:4096], xst[:, 2560:4096])
        for g in range(1, NG):
            dma = nc.sync.dma_start(xtgs[g][:],
                                    xst[:, g * 4096:(g + 1) * 4096])
            if g == 1:
                add_dep_helper(dma.ins, wv_last.ins, sync=True,
                               reason="x groups 1-3 load behind wv")

        # ---------------- Pass 1: q/k/v projections, kv + ksum ----------------
        p1i = ExitStack()
        kpool = p1i.enter_context(tc.tile_pool(name="kp", bufs=5))
        vpool = p1i.enter_context(tc.tile_pool(name="vp", bufs=2))
        projp = p1i.enter_context(tc.tile_pool(name="projp", bufs=4, space="PSUM"))

        def add_bias(ps, boff, g):
            if has_bias:
                nc.tensor.matmul(
                    ps[:],
                    ones_row[0:1, 0:128],
                    bias_sb[0:1, boff + g * 512: boff + g * 512 + 512],
                    start=False, stop=True,
                )

        st = {}
        ksbs = {}

        def kv_matmul(t, ksb, vsb, r):
            # NOTE: start=True clears has_written for the whole PSUM
            # bank (2 blocks), so only the even block per bank sets it
            nc.tensor.matmul(
                kv_ps[:, r * 256: r * 256 + 130],
                ksb[:, r * 128:(r + 1) * 128],
                vsb[:, r * 130: r * 130 + 130],
                start=(t == 0 and r % 2 == 0), stop=(t == NT - 1),
            )

        def kproj_half(t, xtg, g):
            tt = t % GT
            kh = projp.tile([128, 512], F32, tag="proj", name=f"kps{t}_{g}")
            for c in range(8):
                if t == 0 and g == 0 and c == 0:
                    # first matmul split so it only waits for the first
                    # 128 x columns + wk's first 512 cols
                    for pc in range(2):
                        nc.tensor.matmul(
                            kh[:, pc * 256:(pc + 1) * 256],
                            xtg[:, 0:128],
                            wk_t[:, pc * 256:(pc + 1) * 256],
                            start=(pc == 0), stop=False)
                    continue
                nc.tensor.matmul(
                    kh[:], xtg[:, c * 512 + tt * 128: c * 512 + tt * 128 + 128],
                    wslice(wk_t, g, c),
                    start=(c == 0), stop=(c == 7 and not has_bias))
            add_bias(kh, 1024, g)
            return kh

        def stage_k(t, xtg):
            # k projection for one tile; elu on DVE/ACT overlaps the
            # following matmuls
            ksb = kpool.tile([128, 1024], mm_dt, tag="k", name=f"ksb{t}")
            ksbs[t] = ksb
            khalves = [kproj_half(t, xtg, g) for g in range(2)]
            for g in range(2):
                elu1_half(ksb[:, g * 512:(g + 1) * 512], khalves[g][:])

        def stage_v(t, xtg):
            # v projection; the 8 kv matmuls of tile t-1 interleave
            # between the two v half-projections so their LDWEIGHTS
            # overlap 512-row matmuls instead of exposing ~100ns each
            tt = t % GT
            vsb = vpool.tile([128, 1040], mm_dt, tag="v", name=f"vsb{t}")
            pv = st.pop(t - 1, None)
            for g in range(2):
                vh = projp.tile([128, 512], F32, tag="proj", name=f"vps{t}_{g}")
                for c in range(8):
                    nc.tensor.matmul(
                        vh[:], xtg[:, c * 512 + tt * 128: c * 512 + tt * 128 + 128],
                        wslice(wv_t, g, c),
                        start=(c == 0), stop=(c == 7 and not has_bias))
                if pv is not None:
                    for r in range(4):
                        kv_matmul(t - 1, pv[0], pv[1], g * 4 + r)
                add_bias(vh, 2048, g)
                # strided copy into the [v | 1] augmented layout, on ACT
                # (the DVE is the pass-1 straggler with the elu chains)
                nc.scalar.activation(
                    vsb[:, g * 520: g * 520 + 520]
                    .rearrange("p (h e) -> p h e", e=65)[:, :, 0:64],
                    vh[:].rearrange("p (h e) -> p h e", e=64),
                    AF.Copy)
            nc.vector.memset(
                vsb[:].rearrange("p (h e) -> p h e", e=65)[:, :, 64:65], 1.0)
            st[t] = (ksbs.pop(t), vsb)

        def stage_b(t):
            ksb, vsb = st.pop(t)
            for r in range(8):
                kv_matmul(t, ksb, vsb, r)

        def send_kv():
            # PSUM f32 -> bf16 in the PSUM-native block layout via the
            # scalar engine (the DVE queue is full of elu work), then
            # one DMA to the collective input
            ccsb = cpool.tile([128, 1040], mm_dt, tag="ccsb")
            nc.scalar.activation(
                ccsb[:].rearrange("p (r w) -> p r w", w=130),
                kv_ps[:].rearrange("p (r w) -> p r w", w=256)[:, :, 0:130],
                AF.Copy)
            nc.sync.dma_start(cc_in[:], ccsb[:])
            nc.gpsimd.collective_compute(
                "AllReduce", mybir.AluOpType.add,
                replica_groups=[[0, 1], [2, 3], [4, 5], [6, 7]],
                ins=[cc_in[:]], outs=[cc_out[:]],
            )

        # warmup: all 8 group-0 k half-projections before any stage_v,
        # g0 halves first (they only need wk piece 0 + x group 0)
        for t in range(GT):
            ksbs[t] = kpool.tile([128, 1024], mm_dt, tag="k", name=f"ksb{t}")
        for g in range(2):
            for t in range(GT):
                kh = kproj_half(t, xtgs[0], g)
                elu1_half(ksbs[t][:, g * 512:(g + 1) * 512], kh[:])
        for t in range(GT):
            stage_v(t, xtgs[0])
        for g in range(1, NG):
            for tt in range(GT):
                t = g * GT + tt
                stage_k(t, xtgs[g])
                stage_v(t, xtgs[g])
        # finish kv, launch the AllReduce, THEN the q^T groups (~55us of
        # matmuls) hide the collective latency
        stage_b(NT - 1)
        send_kv()
        p1i.close()
        kvstack.close()

        # fill bd as soon as the CC lands: rows 0:64 = head 2c (d), rows
        # 64:128 = head 2c+1; cols c*256+[0:64] = kv_2c, [64:128] =
        # kv_2c+1, 128/129 = ksums (sync queue is idle here)
        ccr_lo = cc_out[0:64, :].rearrange("p (c w) -> p c w", w=130)
        ccr_hi = cc_out[64:128, :].rearrange("p (c w) -> p c w", w=130)
        bd_lo = bd[0:64, :].rearrange("p (c r) -> p c r", r=256)
        bd_hi = bd[64:128, :].rearrange("p (c r) -> p c r", r=256)
        nc.sync.dma_start(bd_lo[:, :, 0:64], ccr_lo[:, :, 0:64])
        nc.sync.dma_start(bd_hi[:, :, 64:128], ccr_hi[:, :, 65:129])
        nc.sync.dma_start(bd_lo[:, :, 128:129], ccr_lo[:, :, 64:65])
        nc.sync.dma_start(bd_hi[:, :, 129:130], ccr_hi[:, :, 129:130])

        # od pools live from the q^T phase (attn prefill) to the end;
        # banks 0/1 double-buffered, bank 2 single
        odp2 = top.enter_context(tc.tile_pool(name="odp2", bufs=2, space="PSUM"))
        odp1 = top.enter_context(tc.tile_pool(name="odp1", bufs=1, space="PSUM"))

        def attn_tile(t):
            g, tt = t // GT, t % GT
            qtsb = qts[g]
            # 8 [128,130] blocks packed 3-per-bank: block r = heads
            # (2r, 2r+1); cols 0:128 numerator, 128:130 denominators
            ods = [odp2.tile([128, 512], F32, tag=f"od{b}", name=f"od{t}_{b}")
                   for b in range(2)]
            ods.append(odp1.tile([128, 512], F32, tag="od2", name=f"od{t}_2"))
            zden = zpool.tile([128, 16], F32, tag="zden")
            for r in range(8):
                b, s = r // 3, r % 3
                nc.tensor.matmul(
                    ods[b][:, s * 130: s * 130 + 130],
                    qtsb[:, r * 512 + tt * 128: r * 512 + tt * 128 + 128],
                    bd[:, r * 256: r * 256 + 130],
                    start=(s == 0), stop=(s == NBLK[b] - 1),
                )
            # DVE processes bank 2 FIRST: it is the only single-buffered
            # od bank, and the next tile touches it last (blocks 6-7)
            for b in (2, 0, 1):
                ns = NBLK[b]
                od_r = ods[b][:, 0:ns * 130].rearrange("p (s w) -> p s w", w=130)
                nc.vector.tensor_scalar_add(
                    zden[:, b * 6: b * 6 + 2 * ns]
                    .rearrange("p (s i) -> p s i", i=2),
                    od_r[:, :, 128:130], EPS)
            zinv = zpool.tile([128, 16], F32, tag="zinv")
            nc.vector.reciprocal(zinv[:], zden[:])
            osb = ospool.tile([128, 1024], mm_dt, tag="osb")
            osbs[t] = osb
            for b in (2, 0, 1):
                ns = NBLK[b]
                od_r = ods[b][:, 0:ns * 130].rearrange("p (s w) -> p s w", w=130)
                zb = (zinv[:, b * 6: b * 6 + 2 * ns]
                      .rearrange("p (s i) -> p s i", i=2)
                      .unsqueeze(3).broadcast_to((128, ns, 2, 64)))
                nc.vector.tensor_mul(
                    osb[:, b * 384: b * 384 + ns * 128]
                    .rearrange("p (s i e) -> p s i e", i=2, e=64),
                    od_r[:, :, 0:128].rearrange("p s (i e) -> p s i e", i=2),
                    zb,
                )

        # --- q^T projections (hide the AllReduce), attn tiles 0-1
        # interleaved so their DVE chains drain before group 3's elus ---
        p1q = ExitStack()
        qhp = p1q.enter_context(tc.tile_pool(name="qhp", bufs=3, space="PSUM"))

        def stage_a2(g, xtg):
            qtsb = qtpool.tile([128, 4096], mm_dt, tag=f"qt{g}")
            qts[g] = qtsb
            for dqc in range(8):
                qh = qhp.tile([128, 512], F32, tag="qh", name=f"qps{g}_{dqc}")
                for dc in range(8):
                    nc.tensor.matmul(
                        qh[:],
                        wslice(wq_t, dqc // 4, dc, (dqc % 4) * 128, 128),
                        xtg[:, dc * 512:(dc + 1) * 512],
                        start=(dc == 0), stop=(dc == 7 and not has_bias))
                if has_bias:
                    # q^T bias: bq along partitions = rank-1 with ones row
                    nc.tensor.matmul(
                        qh[:],
                        bias_sb[0:1, dqc * 128: dqc * 128 + 128],
                        ones_row512[0:1, 0:512],
                        start=False, stop=True)
                elu1_half(qtsb[:, dqc * 512:(dqc + 1) * 512], qh[:])

        for g in range(NG):
            stage_a2(g, xtgs[g])
        # attn tiles 0-1 pre-roll AFTER every q^T group: placing them
        # earlier (between groups) exposes the PE to cross-core launch
        # skew — the bd fill waits on the AllReduce, and a lagging
        # partner core stalled the whole queue for tens of us here.
        # With od banks 0/1 double-buffered the fill bubbles are small.
        attn_tile(0)
        attn_tile(1)
        p1q.close()
        p1o.close()

        # ------- Pass 2: attention + normalize + output projection, fused -----
        with ExitStack() as p2:
            y_pool = p2.enter_context(tc.tile_pool(name="ysb", bufs=2))
            # y halves single-buffered (the per-half copies drain early
            # enough), transpose bank single: od 5 + y 2 + ot 1 = 8 banks
            ypp = p2.enter_context(tc.tile_pool(name="ypp", bufs=1, space="PSUM"))
            otpp = p2.enter_context(tc.tile_pool(name="otp", bufs=1, space="PSUM"))

            otbs = {}

            def pe_transpose(t):
                # transpose off the DMA engines: 8 is_transpose matmuls into
                # one bf16 PSUM bank (disjoint regions; start only on the
                # first, so later blocks land on the pending-zero region),
                # then the DVE copies it back to SBUF (GpSimd cannot read
                # PSUM)
                osb = osbs.pop(t)
                otp = otpp.tile([128, 1024], mm_dt, tag="otp", name=f"otp{t}")
                for c in range(8):
                    nc.tensor.matmul(
                        otp[:, c * 128:(c + 1) * 128],
                        osb[:, c * 128:(c + 1) * 128],
                        identm[:],
                        is_transpose=True,
                        start=(c == 0), stop=(c == 7),
                    )
                otb = otpool.tile([128, 1024], mm_dt, tag="otb")
                otbs[t] = otb
                nc.vector.tensor_copy(otb[:], otp[:])

            def wo_tile(t):
                otb = otbs.pop(t)
                for g in range(2):
                    yps = ypp.tile([128, 512], F32, tag=f"y{g}", name=f"yps{t}_{g}")
                    for c in range(8):
                        nc.tensor.matmul(
                            yps[:],
                            otb[:, c * 128:(c + 1) * 128],
                            wslice(wo_t, g, c),
                            start=(c == 0), stop=(c == 7),
                        )
                    # per-half copy+store pipelines the drain: half 0 ships
                    # while half 1's matmuls still run
                    ysb = y_pool.tile([128, 512], mm_dt, tag=f"ysb{g}")
                    if t == NT - 1:
                        # last tile: quarter the copy so the stores start
                        # before the full half is converted
                        for q2 in range(2):
                            nc.scalar.activation(
                                ysb[:, q2 * 256:(q2 + 1) * 256],
                                yps[:, q2 * 256:(q2 + 1) * 256], AF.Copy)
                            eng = nc.sync if q2 == 0 else nc.scalar
                            eng.dma_start(
                                y_d[t * 128:(t + 1) * 128,
                                    g * 512 + q2 * 256: g * 512 + (q2 + 1) * 256],
                                ysb[:, q2 * 256:(q2 + 1) * 256])
                    elif t == NT - 2:
                        nc.scalar.activation(ysb[:], yps[:], AF.Copy)
                        # split the tail stores across engines/queues
                        for q2 in range(2):
                            eng = nc.sync if q2 == 0 else nc.scalar
                            eng.dma_start(
                                y_d[t * 128:(t + 1) * 128,
                                    g * 512 + q2 * 256: g * 512 + (q2 + 1) * 256],
                                ysb[:, q2 * 256:(q2 + 1) * 256])
                    else:
                        nc.scalar.activation(ysb[:], yps[:], AF.Copy)
                        nc.sync.dma_start(
                            y_d[t * 128:(t + 1) * 128, g * 512:(g + 1) * 512],
                            ysb[:])

            LAG = 3   # tiles between attn and wo
            for t in range(2, NT):
                # 2-tile transpose lag so the DVE normalize chain of tile
                # t-2 is surely done and the PE never waits on it; the
                # transpose runs before attn so the attn matmuls give the
                # DVE chain of tile t-1 extra slack
                pe_transpose(t - 2)
                attn_tile(t)
                if t >= LAG:
                    wo_tile(t - LAG)
            pe_transpose(NT - 2)
            pe_transpose(NT - 1)
            for t in range(NT - LAG, NT):
                wo_tile(t)


def _get_program(has_bias):
    key = (has_bias, MM_DT)
    if key not in _PROGRAMS:
        nc = bacc.Bacc("TRN2", target_bir_lowering=False, debug=False,
                       num_devices=N_CORES)
        _emit(nc, has_bias, MM_DT)
        nc.compile()
        _PROGRAMS[key] = nc
    return _PROGRAMS[key]


def _to_mm_np(a):
    """Convert fp32 array to the numpy dtype matching MM_DT."""
    if MM_DT == BF16:
        import ml_dtypes
        return np.ascontiguousarray(a.astype(ml_dtypes.bfloat16))
    return np.ascontiguousarray(a)


def _pack_rhs(w):
    # W [out,in] -> W.T half-major rhs layout [128, 2*8*512]:
    # [p, h*4096 + c*512 + n] = W.T[c*128+p, h*512+n]
    return _to_mm_np(
        w.T.reshape(8, 128, 2, 512).transpose(1, 2, 0, 3).reshape(128, 8 * D))


def _pack_xt(xs):
    # xs [TOK, D] -> x^T group-major: [p, g*4096 + c*512 + u] = xs[g*512+u, c*128+p]
    ng = NT // 4
    return _to_mm_np(
        xs.T.reshape(8, 128, ng, 512).transpose(1, 2, 0, 3).reshape(128, NT * 1024))


def kernel(x, Wq, bq, Wk, bk, Wv, bv, Wo, bo):
    global LAST_RESULT
    x = np.asarray(x, dtype=np.float32)
    Wq, Wk, Wv, Wo = (np.asarray(w, dtype=np.float32) for w in (Wq, Wk, Wv, Wo))
    bq, bk, bv, bo = (np.asarray(b, dtype=np.float32) for b in (bq, bk, bv, bo))

    has_bias = bool(np.any(bq) or np.any(bk) or np.any(bv))
    nc = _get_program(has_bias)
    shared = {
        "wkt": _pack_rhs(Wk),
        "wvt": _pack_rhs(Wv),
        "wqt": _pack_rhs(Wq),
        "wot": _pack_rhs(Wo),
        "ident": _to_mm_np(np.eye(128, dtype=np.float32)),
        "biases": np.concatenate([bq, bk, bv, bo]).reshape(1, 4096),
    }
    in_maps = []
    for c in range(N_CORES):
        b = c // 2
        h = c % 2
        m = dict(shared)
        m["xst"] = _pack_xt(x[b, h * TOK:(h + 1) * TOK, :])
        in_maps.append(m)

    res = run_bass_kernel_spmd(nc, in_maps, list(range(N_CORES)), trace=TRACE)
    LAST_RESULT = res

    y = np.empty((B, S, D), dtype=np.float32)
    for c in range(N_CORES):
        b = c // 2
        h = c % 2
        y[b, h * TOK:(h + 1) * TOK, :] = np.asarray(
            res.results[c]["y"]).astype(np.float32)
    y += bo
    return y


# revision 25
# speedup vs baseline: 1.0169x; 1.0169x over previous
"""Linear self-attention (elu+1 feature map) Trainium2 kernel — bf16.

Problem: B=4, S=4096, D=1024, H=16, HD=64.
  q = elu1(x @ Wq.T + bq); k = elu1(x @ Wk.T + bk); v = x @ Wv.T + bv
  kv_h = k_h^T v_h; ksum_h = sum_t k_h; z = 1/(q.ksum + eps)
  out = (q_h @ kv_h) * z; y = out @ Wo.T + bo
Sharding: token-parallel. Core c handles batch c//2, sequence half c%2
(2048 tokens). kv/ksum are partial sums over local tokens, AllReduced
(bf16) across the 2-core group sharing a batch, then every core
finishes its own tokens through attention + output projection. bo is
added on host.

All PE-facing operands are bf16 (1 cycle/row on the PE; fp32 runs at
2). PSUM accumulation stays fp32. q^T stays resident in SBUF.

Schedule notes:
 - Weights are packed HALF-MAJOR (all chunks' first 512 output dims,
   then all second 512), so one 1MB DMA delivers a full projection
   half. DMA triggers cost ~0.5us of queue time each and queues starve
   for instruction fetch under HBM pressure, so loads are few and
   queue-serial in urgency order: wk (3 pieces) then wv/wq/wo (2 each)
   on gpsimd; x group 0 (4 pieces) then groups 1-3 (1 each, behind wv)
   on sync.
 - Warmup: the 8 group-0 k half-projections run before any
   v-projection, g0 halves first — they need only wk's first 1MB +
   x group 0, so the PE starts ~1us after the preamble and never waits
   for wv.
 - Pass 1 pipeline per tile: k-proj -> v-proj with the previous tile's
   8 kv matmuls interleaved between v half-projections; elu runs on
   DVE/ACT under the next tile's matmuls; the [v|1] assembly copy runs
   on ACT (the DVE is the straggler engine in pass 1).
 - The kv AllReduce launches right after the last kv matmul; the 4 q^T
   projection groups (~55us of matmuls) hide its latency. attn tiles
   0-1 are emitted between q^T groups 2 and 3 so their DVE normalize
   chains drain before the q^T group-3 elu backlog, removing the
   pass-2 pipeline-fill stalls.
 - Pass 2 per tile: 8 attn matmuls with 130-col rhs (the useful
   [qkv | den] block), DVE normalizes (bank 2 first — it is the only
   single-buffered od bank), the PE transposes the normalized output
   via is_transpose matmuls into a bf16 PSUM bank (a DMA transpose
   here costs 256KB/tile of 2-byte-gather SBUF traffic that halves
   concurrent matmul throughput), the DVE copies it back to SBUF, and
   the Wo matmuls consume it 3 tiles later, split into halves so y
   copies/stores pipeline with the drain.
 - PSUM budget: pass 1: kv 4 + proj 4; q^T phase: qh 3 + od 5;
   pass 2: od 5 + y halves 2 + transpose 1 = 8.
"""

import numpy as np
from contextlib import ExitStack

import concourse.bass as bass
import concourse.tile as tile
from concourse import bacc, mybir
from concourse.bass_utils import run_bass_kernel_spmd
from concourse.tile_rust import add_dep_helper

B, S, D, H, HD = 4, 4096, 1024, 16, 64
N_CORES = 8
TOK = (B * S) // N_CORES      # 2048 tokens per core
NT = TOK // 128               # 16 token tiles per core
GT = 4                        # token tiles per x^T group
NG = NT // GT
F32 = mybir.dt.float32
BF16 = mybir.dt.bfloat16
EPS = 1e-6

MM_DT = BF16

TRACE = False            # set by test harness for profiling
LAST_RESULT = None       # BassKernelResults of last run

_PROGRAMS = {}


def _emit(nc, has_bias, mm_dt):
    AF = mybir.ActivationFunctionType
    ALU = mybir.AluOpType

    # x^T, chunk-major per token tile within each group:
    # [p, g*4096 + c*512 + u] = x[g*512 + u, c*128 + p]
    xst = nc.dram_tensor("xst", [128, NT * 1024], mm_dt, kind="ExternalInput").ap()
    # weights half-major: [p, h*4096 + c*512 + n] = W.T[c*128+p, h*512+n]
    wkd = nc.dram_tensor("wkt", [128, 8 * D], mm_dt, kind="ExternalInput").ap()
    wvd = nc.dram_tensor("wvt", [128, 8 * D], mm_dt, kind="ExternalInput").ap()
    wqd = nc.dram_tensor("wqt", [128, 8 * D], mm_dt, kind="ExternalInput").ap()
    wod = nc.dram_tensor("wot", [128, 8 * D], mm_dt, kind="ExternalInput").ap()
    identd = nc.dram_tensor("ident", [128, 128], mm_dt, kind="ExternalInput").ap()
    biasd = nc.dram_tensor("biases", [1, 4096], F32, kind="ExternalInput").ap()
    # y ships bf16 (host upcasts); halves store traffic and the drain tail
    y_d = nc.dram_tensor("y", [TOK, D], mm_dt, kind="ExternalOutput").ap()
    # kv collective payload keeps the PSUM block layout: block r (heads
    # 2r, 2r+1) at cols r*130; rows 0:64 x 0:65 = [kv_2r | ksum_2r],
    # rows 64:128 x 65:130 = [kv_2r+1 | ksum_2r+1] (complement is junk)
    cc_in = nc.dram_tensor("cc_in", [128, 1040], mm_dt).ap()
    cc_out = nc.dram_tensor("cc_out", [128, 1040], mm_dt).ap()

    def wslice(wt, g, c, n0=0, nn=512):
        # half-major weight slice: out-half g, chunk c, cols n0:n0+nn
        return wt[:, g * 4096 + c * 512 + n0: g * 4096 + c * 512 + n0 + nn]

    with tile.TileContext(nc) as tc, ExitStack() as top:
        wpool = top.enter_context(tc.tile_pool(name="w", bufs=4))
        cpool = top.enter_context(tc.tile_pool(name="const", bufs=1))
        qtpool = top.enter_context(tc.tile_pool(name="qt", bufs=1))
        ospool = top.enter_context(tc.tile_pool(name="os", bufs=4))
        otpool = top.enter_context(tc.tile_pool(name="ot", bufs=5))
        zpool = top.enter_context(tc.tile_pool(name="z", bufs=2))
        # identity for the pass-2 PE transposes
        identm = cpool.tile([128, 128], mm_dt, tag="ident")
        nc.scalar.dma_start(identm[:], identd)
        # block-diagonal [kv | ksum] matrix for pass 2 (chunk c = heads
        # 2c, 2c+1); zeroed now while the DVE is idle, filled after the CC
        bd = cpool.tile([128, 2048], mm_dt, tag="bd")
        nc.vector.memset(bd[:].bitcast(F32), 0.0)
        if has_bias:
            ones_row_st = cpool.tile([1, 512], F32, tag="ones_row_st")
            nc.vector.memset(ones_row_st[:], 1.0)
            ones_row = cpool.tile([1, 128], mm_dt, tag="ones_row")
            nc.vector.tensor_copy(ones_row[:], ones_row_st[0:1, 0:128])
            ones_row512 = cpool.tile([1, 512], mm_dt, tag="ones_row512")
            nc.vector.tensor_copy(ones_row512[:], ones_row_st[:])
            bias_st = cpool.tile([1, 3072], F32, tag="bias_st")
            nc.sync.dma_start(bias_st[:], biasd[0:1, 0:3072])
            bias_sb = cpool.tile([1, 3072], mm_dt, tag="bias")
            nc.vector.tensor_copy(bias_sb[:], bias_st[:])

        # --- weight loads: few large DMAs, queue-serial on gpsimd in
        # urgency order; half-major layout means piece 0 serves all g=0
        # half-projections ---
        wk_t = wpool.tile([128, 8 * D], mm_dt, tag="w", name="wk")
        # finer pieces: warmup consumes wk chunk-by-chunk at ~450GB/s, so
        # lumpy arrival directly stalls the PE
        for lo, hi in ((0, 256), (256, 512), (512, 1024), (1024, 2048),
                       (2048, 3072), (3072, 4096), (4096, 6144), (6144, 8192)):
            nc.gpsimd.dma_start(wk_t[:, lo:hi], wkd[:, lo:hi])

        def load_weight_big(dram_ap, name):
            wt = wpool.tile([128, 8 * D], mm_dt, tag="w", name=name)
            last = None
            for hf in range(2):
                last = nc.gpsimd.dma_start(
                    wt[:, hf * 4096:(hf + 1) * 4096],
                    dram_ap[:, hf * 4096:(hf + 1) * 4096])
            return wt, last

        wv_t, wv_last = load_weight_big(wvd, "wv")
        wq_t, _ = load_weight_big(wqd, "wq")
        wo_t, _ = load_weight_big(wod, "wo")

        kvstack = ExitStack()
        kvpool = kvstack.enter_context(tc.tile_pool(name="kvp", bufs=1, space="PSUM"))
        # 2-head-batched kv accumulator: block r (heads 2r, 2r+1) at cols
        # r*256: rows 0:64 x cols 0:65 = [kv_2r | ksum_2r], rows 64:128 x
        # cols 65:130 = [kv_2r+1 | ksum_2r+1]; the other corners are unused
        kv_ps = kvpool.tile([128, 2048], F32, tag="kv")

        qts = {}
        osbs = {}
        NBLK = (3, 3, 2)   # attn blocks per PSUM bank (8 = 3+3+2)

        p1o = ExitStack()
        xtpool = p1o.enter_context(tc.tile_pool(name="xt", bufs=4))
        mepool = p1o.enter_context(tc.tile_pool(name="me", bufs=4))

        def elu1_half(dst_half, ps_half):
            # elu(x)+1 = exp(min(x,0)) + max(x,0), on a [128,512] half
            me = mepool.tile([128, 512], F32, tag="me")
            nc.vector.tensor_scalar_min(me[:], ps_half, 0.0)
            nc.scalar.activation(me[:], me[:], AF.Exp)
            nc.vector.scalar_tensor_tensor(
                dst_half, ps_half, 0.0, me[:], ALU.max, ALU.add)

        # x loads: group 0 in 4 pieces (first 128 cols split off so matmul
        # 0 starts sooner); groups 1-3 single 1MB DMAs, queue-serial on
        # sync, group 1 deferred behind wv so early HBM serves wk+x0+wv —
        # the bytes the PE actually waits for
        xtgs = {}
        for g in range(NG):
            xtgs[g] = xtpool.tile([128, GT * 1024], mm_dt, tag="xt",
                                  name=f"xtg{g}")
        nc.sync.dma_start(xtgs[0][:, 0:128], xst[:, 0:128])
        nc.sync.dma_start(xtgs[0][:, 128:1024], xst[:, 128:1024])
        nc.sync.dma_start(xtgs[0][:, 1024:2560], xst[:, 1024:2560])
        nc.sync.dma_start(xtgs[0][:, 2560:4096], xst[:, 2560:4096])
        for g in range(1, NG):
            dma = nc.sync.dma_start(xtgs[g][:],
                                    xst[:, g * 4096:(g + 1) * 4096])
            if g == 1:
                add_dep_helper(dma.ins, wv_last.ins, sync=True,
                               reason="x groups 1-3 load behind wv")

        # ---------------- Pass 1: q/k/v projections, kv + ksum ----------------
        p1i = ExitStack()
        kpool = p1i.enter_context(tc.tile_pool(name="kp", bufs=5))
        vpool = p1i.enter_context(tc.tile_pool(name="vp", bufs=2))
        projp = p1i.enter_context(tc.tile_pool(name="projp", bufs=4, space="PSUM"))

        def add_bias(ps, boff, g):
            if has_bias:
                nc.tensor.matmul(
                    ps[:],
                    ones_row[0:1, 0:128],
                    bias_sb[0:1, boff + g * 512: boff + g * 512 + 512],
                    start=False, stop=True,
                )

        st = {}
        ksbs = {}

        def kv_matmul(t, ksb, vsb, r):
            # NOTE: start=True clears has_written for the whole PSUM
            # bank (2 blocks), so only the even block per bank sets it
            nc.tensor.matmul(
                kv_ps[:, r * 256: r * 256 + 130],
                ksb[:, r * 128:(r + 1) * 128],
                vsb[:, r * 130: r * 130 + 130],
                start=(t == 0 and r % 2 == 0), stop=(t == NT - 1),
            )

        def kproj_half(t, xtg, g):
            tt = t % GT
            kh = projp.tile([128, 512], F32, tag="proj", name=f"kps{t}_{g}")
            for c in range(8):
                if t == 0 and g == 0 and c == 0:
                    # first matmul split so it only waits for the first
                    # 128 x columns + wk's first 512 cols
                    for pc in range(2):
                        nc.tensor.matmul(
                            kh[:, pc * 256:(pc + 1) * 256],
                            xtg[:, 0:128],
                            wk_t[:, pc * 256:(pc + 1) * 256],
                            start=(pc == 0), stop=False)
                    continue
                nc.tensor.matmul(
                    kh[:], xtg[:, c * 512 + tt * 128: c * 512 + tt * 128 + 128],
                    wslice(wk_t, g, c),
                    start=(c == 0), stop=(c == 7 and not has_bias))
            add_bias(kh, 1024, g)
            return kh

        def stage_k(t, xtg):
            # k projection for one tile; elu on DVE/ACT overlaps the
            # following matmuls
            ksb = kpool.tile([128, 1024], mm_dt, tag="k", name=f"ksb{t}")
            ksbs[t] = ksb
            khalves = [kproj_half(t, xtg, g) for g in range(2)]
            for g in range(2):
                elu1_half(ksb[:, g * 512:(g + 1) * 512], khalves[g][:])

        def stage_v(t, xtg):
            # v projection; the 8 kv matmuls of tile t-1 interleave
            # between the two v half-projections so their LDWEIGHTS
            # overlap 512-row matmuls instead of exposing ~100ns each
            tt = t % GT
            vsb = vpool.tile([128, 1040], mm_dt, tag="v", name=f"vsb{t}")
            pv = st.pop(t - 1, None)
            for g in range(2):
                vh = projp.tile([128, 512], F32, tag="proj", name=f"vps{t}_{g}")
                for c in range(8):
                    nc.tensor.matmul(
                        vh[:], xtg[:, c * 512 + tt * 128: c * 512 + tt * 128 + 128],
                        wslice(wv_t, g, c),
                        start=(c == 0), stop=(c == 7 and not has_bias))
                if pv is not None:
                    for r in range(4):
                        kv_matmul(t - 1, pv[0], pv[1], g * 4 + r)
                add_bias(vh, 2048, g)
                # strided copy into the [v | 1] augmented layout, on ACT
                # (the DVE is the pass-1 straggler with the elu chains)
                nc.scalar.activation(
                    vsb[:, g * 520: g * 520 + 520]
                    .rearrange("p (h e) -> p h e", e=65)[:, :, 0:64],
                    vh[:].rearrange("p (h e) -> p h e", e=64),
                    AF.Copy)
            nc.vector.memset(
                vsb[:].rearrange("p (h e) -> p h e", e=65)[:, :, 64:65], 1.0)
            st[t] = (ksbs.pop(t), vsb)

        def stage_b(t):
            ksb, vsb = st.pop(t)
            for r in range(8):
                kv_matmul(t, ksb, vsb, r)

        def send_kv():
            # PSUM f32 -> bf16 in the PSUM-native block layout via the
            # scalar engine (the DVE queue is full of elu work), then
            # one DMA to the collective input
            ccsb = cpool.tile([128, 1040], mm_dt, tag="ccsb")
            nc.scalar.activation(
                ccsb[:].rearrange("p (r w) -> p r w", w=130),
                kv_ps[:].rearrange("p (r w) -> p r w", w=256)[:, :, 0:130],
                AF.Copy)
            nc.sync.dma_start(cc_in[:], ccsb[:])
            nc.gpsimd.collective_compute(
                "AllReduce", mybir.AluOpType.add,
                replica_groups=[[0, 1], [2, 3], [4, 5], [6, 7]],
                ins=[cc_in[:]], outs=[cc_out[:]],
            )

        # warmup: all 8 group-0 k half-projections before any stage_v,
        # g0 halves first (they only need wk piece 0 + x group 0)
        for t in range(GT):
            ksbs[t] = kpool.tile([128, 1024], mm_dt, tag="k", name=f"ksb{t}")
        for g in range(2):
            for t in range(GT):
                kh = kproj_half(t, xtgs[0], g)
                elu1_half(ksbs[t][:, g * 512:(g + 1) * 512], kh[:])
        for t in range(GT):
            stage_v(t, xtgs[0])
        for g in range(1, NG):
            for tt in range(GT):
                t = g * GT + tt
                stage_k(t, xtgs[g])
                stage_v(t, xtgs[g])
        # finish kv, launch the AllReduce, THEN the q^T groups (~55us of
        # matmuls) hide the collective latency
        stage_b(NT - 1)
        send_kv()
        p1i.close()
        kvstack.close()

        # fill bd as soon as the CC lands: rows 0:64 = head 2c (d), rows
        # 64:128 = head 2c+1; cols c*256+[0:64] = kv_2c, [64:128] =
        # kv_2c+1, 128/129 = ksums (sync queue is idle here)
        ccr_lo = cc_out[0:64, :].rearrange("p (c w) -> p c w", w=130)
        ccr_hi = cc_out[64:128, :].rearrange("p (c w) -> p c w", w=130)
        bd_lo = bd[0:64, :].rearrange("p (c r) -> p c r", r=256)
        bd_hi = bd[64:128, :].rearrange("p (c r) -> p c r", r=256)
        nc.sync.dma_start(bd_lo[:, :, 0:64], ccr_lo[:, :, 0:64])
        nc.sync.dma_start(bd_hi[:, :, 64:128], ccr_hi[:, :, 65:129])
        nc.sync.dma_start(bd_lo[:, :, 128:129], ccr_lo[:, :, 64:65])
        nc.sync.dma_start(bd_hi[:, :, 129:130], ccr_hi[:, :, 129:130])

        # od pools live from the q^T phase (attn prefill) to the end;
        # banks 0/1 double-buffered, bank 2 single
        odp2 = top.enter_context(tc.tile_pool(name="odp2", bufs=2, space="PSUM"))
        odp1 = top.enter_context(tc.tile_pool(name="odp1", bufs=1, space="PSUM"))

        def attn_tile(t):
            g, tt = t // GT, t % GT
            qtsb = qts[g]
            # 8 [128,130] blocks packed 3-per-bank: block r = heads
            # (2r, 2r+1); cols 0:128 numerator, 128:130 denominators
            ods = [odp2.tile([128, 512], F32, tag=f"od{b}", name=f"od{t}_{b}")
                   for b in range(2)]
            ods.append(odp1.tile([128, 512], F32, tag="od2", name=f"od{t}_2"))
            zden = zpool.tile([128, 16], F32, tag="zden")
            for r in range(8):
                b, s = r // 3, r % 3
                nc.tensor.matmul(
                    ods[b][:, s * 130: s * 130 + 130],
                    qtsb[:, r * 512 + tt * 128: r * 512 + tt * 128 + 128],
                    bd[:, r * 256: r * 256 + 130],
                    start=(s == 0), stop=(s == NBLK[b] - 1),
                )
            # DVE processes bank 2 FIRST: it is the only single-buffered
            # od bank, and the next tile touches it last (blocks 6-7)
            for b in (2, 0, 1):
                ns = NBLK[b]
                od_r = ods[b][:, 0:ns * 130].rearrange("p (s w) -> p s w", w=130)
                nc.vector.tensor_scalar_add(
                    zden[:, b * 6: b * 6 + 2 * ns]
                    .rearrange("p (s i) -> p s i", i=2),
                    od_r[:, :, 128:130], EPS)
            zinv = zpool.tile([128, 16], F32, tag="zinv")
            nc.vector.reciprocal(zinv[:], zden[:])
            osb = ospool.tile([128, 1024], mm_dt, tag="osb")
            osbs[t] = osb
            for b in (2, 0, 1):
                ns = NBLK[b]
                od_r = ods[b][:, 0:ns * 130].rearrange("p (s w) -> p s w", w=130)
                zb = (zinv[:, b * 6: b * 6 + 2 * ns]
                      .rearrange("p (s i) -> p s i", i=2)
                      .unsqueeze(3).broadcast_to((128, ns, 2, 64)))
                nc.vector.tensor_mul(
                    osb[:, b * 384: b * 384 + ns * 128]
                    .rearrange("p (s i e) -> p s i e", i=2, e=64),
                    od_r[:, :, 0:128].rearrange("p s (i e) -> p s i e", i=2),
                    zb,
                )

        # --- q^T projections (hide the AllReduce), attn tiles 0-1
        # interleaved so their DVE chains drain before group 3's elus ---
        p1q = ExitStack()
        qhp = p1q.enter_context(tc.tile_pool(name="qhp", bufs=3, space="PSUM"))

        def stage_a2(g, xtg, defer_tail_elus=False):
            qtsb = qtpool.tile([128, 4096], mm_dt, tag=f"qt{g}")
            qts[g] = qtsb
            deferred = []
            for dqc in range(8):
                qh = qhp.tile([128, 512], F32, tag="qh", name=f"qps{g}_{dqc}")
                for dc in range(8):
                    nc.tensor.matmul(
                        qh[:],
                        wslice(wq_t, dqc // 4, dc, (dqc % 4) * 128, 128),
                        xtg[:, dc * 512:(dc + 1) * 512],
                        start=(dc == 0), stop=(dc == 7 and not has_bias))
                if has_bias:
                    # q^T bias: bq along partitions = rank-1 with ones row
                    nc.tensor.matmul(
                        qh[:],
                        bias_sb[0:1, dqc * 128: dqc * 128 + 128],
                        ones_row512[0:1, 0:512],
                        start=False, stop=True)
                if defer_tail_elus and dqc >= 5:
                    # the last 3 qh tiles have no PSUM-reuse successor, so
                    # their elus can run after the attn pre-roll chains —
                    # keeping the DVE free for mul(0)/mul(1)
                    deferred.append((qtsb[:, dqc * 512:(dqc + 1) * 512], qh))
                else:
                    elu1_half(qtsb[:, dqc * 512:(dqc + 1) * 512], qh[:])
            return deferred

        # attn tiles 0-1 pre-roll AFTER every q^T group: placing them
        # earlier (between groups) exposes the PE to cross-core launch
        # skew — the bd fill waits on the AllReduce, and a lagging
        # partner core stalled the whole queue for tens of us here.
        # With od banks 0/1 double-buffered the fill bubbles are small.
        for g in range(NG - 1):
            stage_a2(g, xtgs[g])
        deferred = stage_a2(NG - 1, xtgs[NG - 1], defer_tail_elus=True)
        attn_tile(0)
        attn_tile(1)
        for dst, qh in deferred:
            elu1_half(dst, qh[:])
        p1q.close()
        p1o.close()

        # ------- Pass 2: attention + normalize + output projection, fused -----
        with ExitStack() as p2:
            y_pool = p2.enter_context(tc.tile_pool(name="ysb", bufs=2))
            # y halves single-buffered (the per-half copies drain early
            # enough), transpose bank single: od 5 + y 2 + ot 1 = 8 banks
            ypp = p2.enter_context(tc.tile_pool(name="ypp", bufs=1, space="PSUM"))
            otpp = p2.enter_context(tc.tile_pool(name="otp", bufs=1, space="PSUM"))

            otbs = {}

            def pe_transpose(t):
                # transpose off the DMA engines: 8 is_transpose matmuls into
                # one bf16 PSUM bank (disjoint regions; start only on the
                # first, so later blocks land on the pending-zero region),
                # then the DVE copies it back to SBUF (GpSimd cannot read
                # PSUM)
                osb = osbs.pop(t)
                otp = otpp.tile([128, 1024], mm_dt, tag="otp", name=f"otp{t}")
                # chunk order follows osb readiness (the DVE muls run bank
                # 2 = chunks 6,7 first), so at pipeline fill the first
                # transposes need not wait for the whole normalize chain
                for i, c in enumerate((6, 7, 0, 1, 2, 3, 4, 5)):
                    nc.tensor.matmul(
                        otp[:, c * 128:(c + 1) * 128],
                        osb[:, c * 128:(c + 1) * 128],
                        identm[:],
                        is_transpose=True,
                        start=(i == 0), stop=(i == 7),
                    )
                otb = otpool.tile([128, 1024], mm_dt, tag="otb")
                otbs[t] = otb
                nc.vector.tensor_copy(otb[:], otp[:])

            def wo_tile(t):
                otb = otbs.pop(t)
                for g in range(2):
                    yps = ypp.tile([128, 512], F32, tag=f"y{g}", name=f"yps{t}_{g}")
                    for c in range(8):
                        nc.tensor.matmul(
                            yps[:],
                            otb[:, c * 128:(c + 1) * 128],
                            wslice(wo_t, g, c),
                            start=(c == 0), stop=(c == 7),
                        )
                    # per-half copy+store pipelines the drain: half 0 ships
                    # while half 1's matmuls still run
                    ysb = y_pool.tile([128, 512], mm_dt, tag=f"ysb{g}")
                    if t == NT - 1:
                        # last tile: quarter the copy so the stores start
                        # before the full half is converted (finer splits
                        # lose: each DMA trigger costs ~0.5us of queue time)
                        for q2 in range(2):
                            nc.scalar.activation(
                                ysb[:, q2 * 256:(q2 + 1) * 256],
                                yps[:, q2 * 256:(q2 + 1) * 256], AF.Copy)
                            eng = nc.sync if q2 == 0 else nc.scalar
                            eng.dma_start(
                                y_d[t * 128:(t + 1) * 128,
                                    g * 512 + q2 * 256: g * 512 + (q2 + 1) * 256],
                                ysb[:, q2 * 256:(q2 + 1) * 256])
                    elif t == NT - 2:
                        nc.scalar.activation(ysb[:], yps[:], AF.Copy)
                        # split the tail stores across engines/queues
                        for q2 in range(2):
                            eng = nc.sync if q2 == 0 else nc.scalar
                            eng.dma_start(
                                y_d[t * 128:(t + 1) * 128,
                                    g * 512 + q2 * 256: g * 512 + (q2 + 1) * 256],
                                ysb[:, q2 * 256:(q2 + 1) * 256])
                    else:
                        nc.scalar.activation(ysb[:], yps[:], AF.Copy)
                        nc.sync.dma_start(
                            y_d[t * 128:(t + 1) * 128, g * 512:(g + 1) * 512],
                            ysb[:])

            LAG = 3   # tiles between attn and wo
            for t in range(2, NT):
                # 2-tile transpose lag so the DVE normalize chain of tile
                # t-2 is surely done and the PE never waits on it; the
                # transpose runs before attn so the attn matmuls give the
                # DVE chain of tile t-1 extra slack
                pe_transpose(t - 2)
                attn_tile(t)
                if t >= LAG:
                    wo_tile(t - LAG)
            # drain: a wo tile between the last two transposes hides the
            # DVE normalize latency of tile NT-1
            pe_transpose(NT - 2)
            wo_tile(NT - 3)
            pe_transpose(NT - 1)
            wo_tile(NT - 2)
            wo_tile(NT - 1)


def _get_program(has_bias):
    key = (has_bias, MM_DT)
    if key not in _PROGRAMS:
        nc = bacc.Bacc("TRN2", target_bir_lowering=False, debug=False,
                       num_devices=N_CORES)
        _emit(nc, has_bias, MM_DT)
        nc.compile()
        _PROGRAMS[key] = nc
    return _PROGRAMS[key]


def _to_mm_np(a):
    """Convert fp32 array to the numpy dtype matching MM_DT."""
    if MM_DT == BF16:
        import ml_dtypes
        return np.ascontiguousarray(a.astype(ml_dtypes.bfloat16))
    return np.ascontiguousarray(a)


def _pack_rhs(w):
    # W [out,in] -> W.T half-major rhs layout [128, 2*8*512]:
    # [p, h*4096 + c*512 + n] = W.T[c*128+p, h*512+n]
    return _to_mm_np(
        w.T.reshape(8, 128, 2, 512).transpose(1, 2, 0, 3).reshape(128, 8 * D))


def _pack_xt(xs):
    # xs [TOK, D] -> x^T group-major: [p, g*4096 + c*512 + u] = xs[g*512+u, c*128+p]
    ng = NT // 4
    return _to_mm_np(
        xs.T.reshape(8, 128, ng, 512).transpose(1, 2, 0, 3).reshape(128, NT * 1024))


def kernel(x, Wq, bq, Wk, bk, Wv, bv, Wo, bo):
    global LAST_RESULT
    x = np.asarray(x, dtype=np.float32)
    Wq, Wk, Wv, Wo = (np.asarray(w, dtype=np.float32) for w in (Wq, Wk, Wv, Wo))
    bq, bk, bv, bo = (np.asarray(b, dtype=np.float32) for b in (bq, bk, bv, bo))

    has_bias = bool(np.any(bq) or np.any(bk) or np.any(bv))
    nc = _get_program(has_bias)
    shared = {
        "wkt": _pack_rhs(Wk),
        "wvt": _pack_rhs(Wv),
        "wqt": _pack_rhs(Wq),
        "wot": _pack_rhs(Wo),
        "ident": _to_mm_np(np.eye(128, dtype=np.float32)),
        "biases": np.concatenate([bq, bk, bv, bo]).reshape(1, 4096),
    }
    in_maps = []
    for c in range(N_CORES):
        b = c // 2
        h = c % 2
        m = dict(shared)
        m["xst"] = _pack_xt(x[b, h * TOK:(h + 1) * TOK, :])
        in_maps.append(m)

    res = run_bass_kernel_spmd(nc, in_maps, list(range(N_CORES)), trace=TRACE)
    LAST_RESULT = res

    y = np.empty((B, S, D), dtype=np.float32)
    for c in range(N_CORES):
        b = c // 2
        h = c % 2
        y[b, h * TOK:(h + 1) * TOK, :] = np.asarray(
            res.results[c]["y"]).astype(np.float32)
    y += bo
    return y


# revision 35
# speedup vs baseline: 1.0267x; 1.0097x over previous
"""Linear self-attention (elu+1 feature map) Trainium2 kernel — bf16.

Problem: B=4, S=4096, D=1024, H=16, HD=64.
  q = elu1(x @ Wq.T + bq); k = elu1(x @ Wk.T + bk); v = x @ Wv.T + bv
  kv_h = k_h^T v_h; ksum_h = sum_t k_h; z = 1/(q.ksum + eps)
  out = (q_h @ kv_h) * z; y = out @ Wo.T + bo
Sharding: token-parallel. Core c handles batch c//2, sequence half c%2
(2048 tokens). kv/ksum are partial sums over local tokens, AllReduced
(bf16) across the 2-core group sharing a batch, then every core
finishes its own tokens through attention + output projection. bo is
added on host.

All PE-facing operands are bf16 (1 cycle/row on the PE; fp32 runs at
2). PSUM accumulation stays fp32. q^T stays resident in SBUF.

Schedule notes:
 - Weights are packed HALF-MAJOR (all chunks' first 512 output dims,
   then all second 512), so one 1MB DMA delivers a full projection
   half. DMA triggers cost ~0.5us of queue time each and queues starve
   for instruction fetch under HBM pressure, so loads are few and
   queue-serial in urgency order: wk (3 pieces) then wv/wq/wo (2 each)
   on gpsimd; x group 0 (4 pieces) then groups 1-3 (1 each, behind wv)
   on sync.
 - Warmup: the 8 group-0 k half-projections run before any
   v-projection, g0 halves first — they need only wk's first 1MB +
   x group 0, so the PE starts ~1us after the preamble and never waits
   for wv.
 - Pass 1 pipeline per tile: k-proj -> v-proj with the previous tile's
   8 kv matmuls interleaved between v half-projections; elu runs on
   DVE/ACT under the next tile's matmuls; the [v|1] assembly copy runs
   on ACT (the DVE is the straggler engine in pass 1).
 - The kv AllReduce launches right after the last kv matmul; the 4 q^T
   projection groups (~55us of matmuls) hide its latency. attn tiles
   0-1 are emitted between q^T groups 2 and 3 so their DVE normalize
   chains drain before the q^T group-3 elu backlog, removing the
   pass-2 pipeline-fill stalls.
 - Pass 2 per tile: 8 attn matmuls with 130-col rhs (the useful
   [qkv | den] block), DVE normalizes (bank 2 first — it is the only
   single-buffered od bank), the PE transposes the normalized output
   via is_transpose matmuls into a bf16 PSUM bank (a DMA transpose
   here costs 256KB/tile of 2-byte-gather SBUF traffic that halves
   concurrent matmul throughput), the DVE copies it back to SBUF, and
   the Wo matmuls consume it 3 tiles later, split into halves so y
   copies/stores pipeline with the drain.
 - PSUM budget: pass 1: kv 4 + proj 4; q^T phase: qh 3 + od 5;
   pass 2: od 5 + y halves 2 + transpose 1 = 8.
"""

import numpy as np
from contextlib import ExitStack

import concourse.bass as bass
import concourse.tile as tile
from concourse import bacc, mybir
from concourse.bass_utils import run_bass_kernel_spmd
from concourse.tile_rust import add_dep_helper

B, S, D, H, HD = 4, 4096, 1024, 16, 64
N_CORES = 8
TOK = (B * S) // N_CORES      # 2048 tokens per core
NT = TOK // 128               # 16 token tiles per core
GT = 4                        # token tiles per x^T group
NG = NT // GT
F32 = mybir.dt.float32
BF16 = mybir.dt.bfloat16
EPS = 1e-6

MM_DT = BF16

TRACE = False            # set by test harness for profiling
LAST_RESULT = None       # BassKernelResults of last run

_PROGRAMS = {}


def _emit(nc, has_bias, mm_dt):
    AF = mybir.ActivationFunctionType
    ALU = mybir.AluOpType

    # x^T, chunk-major per token tile within each group:
    # [p, g*4096 + c*512 + u] = x[g*512 + u, c*128 + p]
    xst = nc.dram_tensor("xst", [128, NT * 1024], mm_dt, kind="ExternalInput").ap()
    # weights half-major: [p, h*4096 + c*512 + n] = W.T[c*128+p, h*512+n]
    wkd = nc.dram_tensor("wkt", [128, 8 * D], mm_dt, kind="ExternalInput").ap()
    wvd = nc.dram_tensor("wvt", [128, 8 * D], mm_dt, kind="ExternalInput").ap()
    wqd = nc.dram_tensor("wqt", [128, 8 * D], mm_dt, kind="ExternalInput").ap()
    wod = nc.dram_tensor("wot", [128, 8 * D], mm_dt, kind="ExternalInput").ap()
    identd = nc.dram_tensor("ident", [128, 128], mm_dt, kind="ExternalInput").ap()
    biasd = nc.dram_tensor("biases", [1, 4096], F32, kind="ExternalInput").ap()
    # y ships bf16 (host upcasts); halves store traffic and the drain tail
    y_d = nc.dram_tensor("y", [TOK, D], mm_dt, kind="ExternalOutput").ap()
    # kv collective payload keeps the PSUM block layout: block r (heads
    # 2r, 2r+1) at cols r*130; rows 0:64 x 0:65 = [kv_2r | ksum_2r],
    # rows 64:128 x 65:130 = [kv_2r+1 | ksum_2r+1] (complement is junk)
    cc_in = nc.dram_tensor("cc_in", [128, 1040], mm_dt).ap()
    cc_out = nc.dram_tensor("cc_out", [128, 1040], mm_dt).ap()

    def wslice(wt, g, c, n0=0, nn=512):
        # half-major weight slice: out-half g, chunk c, cols n0:n0+nn
        return wt[:, g * 4096 + c * 512 + n0: g * 4096 + c * 512 + n0 + nn]

    with tile.TileContext(nc) as tc, ExitStack() as top:
        wpool = top.enter_context(tc.tile_pool(name="w", bufs=4))
        cpool = top.enter_context(tc.tile_pool(name="const", bufs=1))
        qtpool = top.enter_context(tc.tile_pool(name="qt", bufs=1))
        ospool = top.enter_context(tc.tile_pool(name="os", bufs=4))
        otpool = top.enter_context(tc.tile_pool(name="ot", bufs=5))
        zpool = top.enter_context(tc.tile_pool(name="z", bufs=2))
        # identity for the pass-2 PE transposes
        identm = cpool.tile([128, 128], mm_dt, tag="ident")
        nc.scalar.dma_start(identm[:], identd)
        # block-diagonal [kv | ksum] matrix for pass 2 (chunk c = heads
        # 2c, 2c+1); zeroed now while the DVE is idle, filled after the CC
        bd = cpool.tile([128, 2048], mm_dt, tag="bd")
        nc.vector.memset(bd[:].bitcast(F32), 0.0)
        if has_bias:
            ones_row_st = cpool.tile([1, 512], F32, tag="ones_row_st")
            nc.vector.memset(ones_row_st[:], 1.0)
            ones_row = cpool.tile([1, 128], mm_dt, tag="ones_row")
            nc.vector.tensor_copy(ones_row[:], ones_row_st[0:1, 0:128])
            ones_row512 = cpool.tile([1, 512], mm_dt, tag="ones_row512")
            nc.vector.tensor_copy(ones_row512[:], ones_row_st[:])
            bias_st = cpool.tile([1, 3072], F32, tag="bias_st")
            nc.sync.dma_start(bias_st[:], biasd[0:1, 0:3072])
            bias_sb = cpool.tile([1, 3072], mm_dt, tag="bias")
            nc.vector.tensor_copy(bias_sb[:], bias_st[:])

        # --- weight loads: few large DMAs, queue-serial on gpsimd in
        # urgency order; half-major layout means piece 0 serves all g=0
        # half-projections ---
        wk_t = wpool.tile([128, 8 * D], mm_dt, tag="w", name="wk")
        # finer pieces: warmup consumes wk chunk-by-chunk at ~450GB/s, so
        # lumpy arrival directly stalls the PE
        for lo, hi in ((0, 256), (256, 512), (512, 1024), (1024, 2048),
                       (2048, 3072), (3072, 4096), (4096, 6144), (6144, 8192)):
            nc.gpsimd.dma_start(wk_t[:, lo:hi], wkd[:, lo:hi])

        def load_weight_big(dram_ap, name):
            wt = wpool.tile([128, 8 * D], mm_dt, tag="w", name=name)
            last = None
            for hf in range(2):
                last = nc.gpsimd.dma_start(
                    wt[:, hf * 4096:(hf + 1) * 4096],
                    dram_ap[:, hf * 4096:(hf + 1) * 4096])
            return wt, last

        wv_t, wv_last = load_weight_big(wvd, "wv")
        wq_t, _ = load_weight_big(wqd, "wq")
        wo_t, _ = load_weight_big(wod, "wo")

        kvstack = ExitStack()
        kvpool = kvstack.enter_context(tc.tile_pool(name="kvp", bufs=1, space="PSUM"))
        # 2-head-batched kv accumulator: block r (heads 2r, 2r+1) at cols
        # r*256: rows 0:64 x cols 0:65 = [kv_2r | ksum_2r], rows 64:128 x
        # cols 65:130 = [kv_2r+1 | ksum_2r+1]; the other corners are unused
        kv_ps = kvpool.tile([128, 2048], F32, tag="kv")

        qts = {}
        osbs = {}
        NBLK = (3, 3, 2)   # attn blocks per PSUM bank (8 = 3+3+2)

        xtpool = top.enter_context(tc.tile_pool(name="xt", bufs=4))
        mepool = top.enter_context(tc.tile_pool(name="me", bufs=4))

        def elu1_half(dst_half, ps_half):
            # elu(x)+1 = exp(min(x,0)) + max(x,0), on a [128,512] half
            me = mepool.tile([128, 512], F32, tag="me")
            nc.vector.tensor_scalar_min(me[:], ps_half, 0.0)
            nc.scalar.activation(me[:], me[:], AF.Exp)
            nc.vector.scalar_tensor_tensor(
                dst_half, ps_half, 0.0, me[:], ALU.max, ALU.add)

        # x loads: group 0 in 4 pieces (first 128 cols split off so matmul
        # 0 starts sooner); groups 1-3 single 1MB DMAs, queue-serial on
        # sync, group 1 deferred behind wv so early HBM serves wk+x0+wv —
        # the bytes the PE actually waits for
        xtgs = {}
        for g in range(NG):
            xtgs[g] = xtpool.tile([128, GT * 1024], mm_dt, tag="xt",
                                  name=f"xtg{g}")
        nc.sync.dma_start(xtgs[0][:, 0:128], xst[:, 0:128])
        nc.sync.dma_start(xtgs[0][:, 128:1024], xst[:, 128:1024])
        nc.sync.dma_start(xtgs[0][:, 1024:2560], xst[:, 1024:2560])
        nc.sync.dma_start(xtgs[0][:, 2560:4096], xst[:, 2560:4096])
        for g in range(1, NG):
            dma = nc.sync.dma_start(xtgs[g][:],
                                    xst[:, g * 4096:(g + 1) * 4096])
            if g == 1:
                add_dep_helper(dma.ins, wv_last.ins, sync=True,
                               reason="x groups 1-3 load behind wv")

        # ---------------- Pass 1: q/k/v projections, kv + ksum ----------------
        p1i = ExitStack()
        kpool = p1i.enter_context(tc.tile_pool(name="kp", bufs=5))
        vpool = p1i.enter_context(tc.tile_pool(name="vp", bufs=2))
        projp = p1i.enter_context(tc.tile_pool(name="projp", bufs=4, space="PSUM"))

        def add_bias(ps, boff, g):
            if has_bias:
                nc.tensor.matmul(
                    ps[:],
                    ones_row[0:1, 0:128],
                    bias_sb[0:1, boff + g * 512: boff + g * 512 + 512],
                    start=False, stop=True,
                )

        st = {}
        ksbs = {}

        def kv_matmul(t, ksb, vsb, r):
            # NOTE: start=True clears has_written for the whole PSUM
            # bank (2 blocks), so only the even block per bank sets it
            nc.tensor.matmul(
                kv_ps[:, r * 256: r * 256 + 130],
                ksb[:, r * 128:(r + 1) * 128],
                vsb[:, r * 130: r * 130 + 130],
                start=(t == 0 and r % 2 == 0), stop=(t == NT - 1),
            )

        def kproj_half(t, xtg, g):
            tt = t % GT
            kh = projp.tile([128, 512], F32, tag="proj", name=f"kps{t}_{g}")
            for c in range(8):
                if t == 0 and g == 0 and c == 0:
                    # first matmul split so it only waits for the first
                    # 128 x columns + wk's first 512 cols
                    for pc in range(2):
                        nc.tensor.matmul(
                            kh[:, pc * 256:(pc + 1) * 256],
                            xtg[:, 0:128],
                            wk_t[:, pc * 256:(pc + 1) * 256],
                            start=(pc == 0), stop=False)
                    continue
                nc.tensor.matmul(
                    kh[:], xtg[:, c * 512 + tt * 128: c * 512 + tt * 128 + 128],
                    wslice(wk_t, g, c),
                    start=(c == 0), stop=(c == 7 and not has_bias))
            add_bias(kh, 1024, g)
            return kh

        def stage_k(t, xtg):
            # k projection for one tile; elu on DVE/ACT overlaps the
            # following matmuls
            ksb = kpool.tile([128, 1024], mm_dt, tag="k", name=f"ksb{t}")
            ksbs[t] = ksb
            khalves = [kproj_half(t, xtg, g) for g in range(2)]
            for g in range(2):
                elu1_half(ksb[:, g * 512:(g + 1) * 512], khalves[g][:])

        def stage_v(t, xtg):
            # v projection; the 8 kv matmuls of tile t-1 interleave
            # between the two v half-projections so their LDWEIGHTS
            # overlap 512-row matmuls instead of exposing ~100ns each
            tt = t % GT
            vsb = vpool.tile([128, 1040], mm_dt, tag="v", name=f"vsb{t}")
            pv = st.pop(t - 1, None)
            for g in range(2):
                vh = projp.tile([128, 512], F32, tag="proj", name=f"vps{t}_{g}")
                for c in range(8):
                    nc.tensor.matmul(
                        vh[:], xtg[:, c * 512 + tt * 128: c * 512 + tt * 128 + 128],
                        wslice(wv_t, g, c),
                        start=(c == 0), stop=(c == 7 and not has_bias))
                if pv is not None:
                    for r in range(4):
                        kv_matmul(t - 1, pv[0], pv[1], g * 4 + r)
                add_bias(vh, 2048, g)
                # strided copy into the [v | 1] augmented layout, on ACT
                # (the DVE is the pass-1 straggler with the elu chains)
                nc.scalar.activation(
                    vsb[:, g * 520: g * 520 + 520]
                    .rearrange("p (h e) -> p h e", e=65)[:, :, 0:64],
                    vh[:].rearrange("p (h e) -> p h e", e=64),
                    AF.Copy)
            nc.vector.memset(
                vsb[:].rearrange("p (h e) -> p h e", e=65)[:, :, 64:65], 1.0)
            st[t] = (ksbs.pop(t), vsb)

        def stage_b(t):
            ksb, vsb = st.pop(t)
            for r in range(8):
                kv_matmul(t, ksb, vsb, r)

        def send_kv():
            # PSUM f32 -> bf16 in the PSUM-native block layout via the
            # scalar engine (the DVE queue is full of elu work), then
            # one DMA to the collective input
            ccsb = cpool.tile([128, 1040], mm_dt, tag="ccsb")
            nc.scalar.activation(
                ccsb[:].rearrange("p (r w) -> p r w", w=130),
                kv_ps[:].rearrange("p (r w) -> p r w", w=256)[:, :, 0:130],
                AF.Copy)
            nc.sync.dma_start(cc_in[:], ccsb[:])
            nc.gpsimd.collective_compute(
                "AllReduce", mybir.AluOpType.add,
                replica_groups=[[0, 1], [2, 3], [4, 5], [6, 7]],
                ins=[cc_in[:]], outs=[cc_out[:]],
            )

        # warmup: all 8 group-0 k half-projections before any stage_v,
        # g0 halves first (they only need wk piece 0 + x group 0).
        # Tiles 0 and 1 interleave chunk-wise: demand per arriving wk
        # chunk halves (2 matmuls instead of 1 tile wanting all 8
        # chunks), matching the HBM arrival rate; tiles 2-3 then run on
        # resident data while the tile-0/1 elus drain, so the g1 pass
        # never waits on PSUM bank recycling
        for t in range(GT):
            ksbs[t] = kpool.tile([128, 1024], mm_dt, tag="k", name=f"ksb{t}")

        def kproj_mm(kh, t, g, c):
            if t == 0 and g == 0 and c == 0:
                for pc in range(2):
                    nc.tensor.matmul(
                        kh[:, pc * 256:(pc + 1) * 256],
                        xtgs[0][:, 0:128],
                        wk_t[:, pc * 256:(pc + 1) * 256],
                        start=(pc == 0), stop=False)
                return
            nc.tensor.matmul(
                kh[:], xtgs[0][:, c * 512 + t * 128: c * 512 + t * 128 + 128],
                wslice(wk_t, g, c),
                start=(c == 0), stop=(c == 7 and not has_bias))

        # 2 tiles interleaved chunk-wise: 3 bunches too many elus at the
        # interleave end, delaying ksb(0) g1 and stalling kv(0) by ~1.4us
        NIL = 2
        khws = [projp.tile([128, 512], F32, tag="proj", name=f"kps{t}_0")
                for t in range(NIL)]
        for c in range(8):
            for t in range(NIL):
                kproj_mm(khws[t], t, 0, c)
        for t in range(NIL):
            add_bias(khws[t], 1024, 0)
            elu1_half(ksbs[t][:, 0:512], khws[t][:])
        for t in range(NIL, GT):
            kh = kproj_half(t, xtgs[0], 0)
            elu1_half(ksbs[t][:, 0:512], kh[:])
        for t in range(GT):
            kh = kproj_half(t, xtgs[0], 1)
            elu1_half(ksbs[t][:, 512:1024], kh[:])
        for t in range(GT):
            stage_v(t, xtgs[0])
        for g in range(1, NG):
            for tt in range(GT):
                t = g * GT + tt
                stage_k(t, xtgs[g])
                stage_v(t, xtgs[g])
        # finish kv, launch the AllReduce, THEN the q^T groups (~55us of
        # matmuls) hide the collective latency
        stage_b(NT - 1)
        send_kv()
        p1i.close()
        kvstack.close()

        # fill bd as soon as the CC lands: rows 0:64 = head 2c (d), rows
        # 64:128 = head 2c+1; cols c*256+[0:64] = kv_2c, [64:128] =
        # kv_2c+1, 128/129 = ksums (sync queue is idle here)
        ccr_lo = cc_out[0:64, :].rearrange("p (c w) -> p c w", w=130)
        ccr_hi = cc_out[64:128, :].rearrange("p (c w) -> p c w", w=130)
        bd_lo = bd[0:64, :].rearrange("p (c r) -> p c r", r=256)
        bd_hi = bd[64:128, :].rearrange("p (c r) -> p c r", r=256)
        nc.sync.dma_start(bd_lo[:, :, 0:64], ccr_lo[:, :, 0:64])
        nc.sync.dma_start(bd_hi[:, :, 64:128], ccr_hi[:, :, 65:129])
        nc.sync.dma_start(bd_lo[:, :, 128:129], ccr_lo[:, :, 64:65])
        nc.sync.dma_start(bd_hi[:, :, 129:130], ccr_hi[:, :, 129:130])

        # od pools live from the q^T phase (attn prefill) to the end;
        # banks 0/1 double-buffered, bank 2 single
        odp2 = top.enter_context(tc.tile_pool(name="odp2", bufs=2, space="PSUM"))
        odp1 = top.enter_context(tc.tile_pool(name="odp1", bufs=1, space="PSUM"))

        def attn_tile(t):
            g, tt = t // GT, t % GT
            qtsb = qts[g]
            # 8 [128,130] blocks packed 3-per-bank: block r = heads
            # (2r, 2r+1); cols 0:128 numerator, 128:130 denominators
            ods = [odp2.tile([128, 512], F32, tag=f"od{b}", name=f"od{t}_{b}")
                   for b in range(2)]
            ods.append(odp1.tile([128, 512], F32, tag="od2", name=f"od{t}_2"))
            zden = zpool.tile([128, 16], F32, tag="zden")
            for r in range(8):
                b, s = r // 3, r % 3
                nc.tensor.matmul(
                    ods[b][:, s * 130: s * 130 + 130],
                    qtsb[:, r * 512 + tt * 128: r * 512 + tt * 128 + 128],
                    bd[:, r * 256: r * 256 + 130],
                    start=(s == 0), stop=(s == NBLK[b] - 1),
                )
            # DVE processes bank 2 FIRST: it is the only single-buffered
            # od bank, and the next tile touches it last (blocks 6-7)
            for b in (2, 0, 1):
                ns = NBLK[b]
                od_r = ods[b][:, 0:ns * 130].rearrange("p (s w) -> p s w", w=130)
                nc.vector.tensor_scalar_add(
                    zden[:, b * 6: b * 6 + 2 * ns]
                    .rearrange("p (s i) -> p s i", i=2),
                    od_r[:, :, 128:130], EPS)
            zinv = zpool.tile([128, 16], F32, tag="zinv")
            nc.vector.reciprocal(zinv[:], zden[:])
            osb = ospool.tile([128, 1024], mm_dt, tag="osb")
            osbs[t] = osb
            for b in (2, 0, 1):
                ns = NBLK[b]
                od_r = ods[b][:, 0:ns * 130].rearrange("p (s w) -> p s w", w=130)
                zb = (zinv[:, b * 6: b * 6 + 2 * ns]
                      .rearrange("p (s i) -> p s i", i=2)
                      .unsqueeze(3).broadcast_to((128, ns, 2, 64)))
                nc.vector.tensor_mul(
                    osb[:, b * 384: b * 384 + ns * 128]
                    .rearrange("p (s i e) -> p s i e", i=2, e=64),
                    od_r[:, :, 0:128].rearrange("p s (i e) -> p s i e", i=2),
                    zb,
                )

        # --- q^T projections (hide the AllReduce), attn tiles 0-1
        # interleaved so their DVE chains drain before group 3's elus ---
        p1q = ExitStack()
        qhp = p1q.enter_context(tc.tile_pool(name="qhp", bufs=3, space="PSUM"))
        defpool = top.enter_context(tc.tile_pool(name="qdef", bufs=3))

        def stage_a2(g, xtg, defer_tail_elus=False):
            qtsb = qtpool.tile([128, 4096], mm_dt, tag=f"qt{g}")
            qts[g] = qtsb
            deferred = []
            for dqc in range(8):
                qh = qhp.tile([128, 512], F32, tag="qh", name=f"qps{g}_{dqc}")
                for dc in range(8):
                    nc.tensor.matmul(
                        qh[:],
                        wslice(wq_t, dqc // 4, dc, (dqc % 4) * 128, 128),
                        xtg[:, dc * 512:(dc + 1) * 512],
                        start=(dc == 0), stop=(dc == 7 and not has_bias))
                if has_bias:
                    # q^T bias: bq along partitions = rank-1 with ones row
                    nc.tensor.matmul(
                        qh[:],
                        bias_sb[0:1, dqc * 128: dqc * 128 + 128],
                        ones_row512[0:1, 0:512],
                        start=False, stop=True)
                if defer_tail_elus and dqc >= 5:
                    # the last 3 qh tiles have no PSUM-reuse successor:
                    # spill them to SBUF on ACT (idle here) and run their
                    # elus one-per-iteration inside the pass-2 loop, where
                    # the DVE has slack — emitting them anywhere around the
                    # attn pre-roll backlogs the DVE right when the first
                    # tiles' normalize chains gate the od-bank recycling
                    qsb = defpool.tile([128, 512], F32, tag="qdef",
                                       name=f"qdef{dqc}")
                    nc.scalar.activation(qsb[:], qh[:], AF.Copy)
                    deferred.append((qtsb[:, dqc * 512:(dqc + 1) * 512], qsb))
                else:
                    elu1_half(qtsb[:, dqc * 512:(dqc + 1) * 512], qh[:])
            return deferred

        # attn tiles 0-1 pre-roll AFTER every q^T group: placing them
        # earlier (between groups) exposes the PE to cross-core launch
        # skew — the bd fill waits on the AllReduce, and a lagging
        # partner core stalled the whole queue for tens of us here.
        # With od banks 0/1 double-buffered the fill bubbles are small.
        for g in range(NG - 1):
            stage_a2(g, xtgs[g])
        deferred = stage_a2(NG - 1, xtgs[NG - 1], defer_tail_elus=True)
        attn_tile(0)
        attn_tile(1)
        p1q.close()

        # ------- Pass 2: attention + normalize + output projection, fused -----
        with ExitStack() as p2:
            y_pool = p2.enter_context(tc.tile_pool(name="ysb", bufs=2))
            # y halves single-buffered (the per-half copies drain early
            # enough), transpose bank single: od 5 + y 2 + ot 1 = 8 banks
            ypp = p2.enter_context(tc.tile_pool(name="ypp", bufs=1, space="PSUM"))
            otpp = p2.enter_context(tc.tile_pool(name="otp", bufs=1, space="PSUM"))

            otbs = {}

            def pe_transpose(t):
                # transpose off the DMA engines: 8 is_transpose matmuls into
                # one bf16 PSUM bank (disjoint regions; start only on the
                # first, so later blocks land on the pending-zero region),
                # then the DVE copies it back to SBUF (GpSimd cannot read
                # PSUM)
                osb = osbs.pop(t)
                otp = otpp.tile([128, 1024], mm_dt, tag="otp", name=f"otp{t}")
                # chunk order follows osb readiness (the DVE muls run bank
                # 2 = chunks 6,7 first), so at pipeline fill the first
                # transposes need not wait for the whole normalize chain
                for i, c in enumerate((6, 7, 0, 1, 2, 3, 4, 5)):
                    nc.tensor.matmul(
                        otp[:, c * 128:(c + 1) * 128],
                        osb[:, c * 128:(c + 1) * 128],
                        identm[:],
                        is_transpose=True,
                        start=(i == 0), stop=(i == 7),
                    )
                otb = otpool.tile([128, 1024], mm_dt, tag="otb")
                otbs[t] = otb
                # on ACT: the DVE queue carries the normalize chains that
                # gate od-bank recycling — keeping this copy off it gets
                # mul(t) done ~0.7us earlier per tile
                nc.scalar.activation(otb[:], otp[:], AF.Copy)

            def wo_tile(t):
                otb = otbs.pop(t)
                for g in range(2):
                    yps = ypp.tile([128, 512], F32, tag=f"y{g}", name=f"yps{t}_{g}")
                    for c in range(8):
                        nc.tensor.matmul(
                            yps[:],
                            otb[:, c * 128:(c + 1) * 128],
                            wslice(wo_t, g, c),
                            start=(c == 0), stop=(c == 7),
                        )
                    # per-half copy+store pipelines the drain: half 0 ships
                    # while half 1's matmuls still run
                    ysb = y_pool.tile([128, 512], mm_dt, tag=f"ysb{g}")
                    if t == NT - 1:
                        # last tile: quarter the copy so the stores start
                        # before the full half is converted (finer splits
                        # lose: each DMA trigger costs ~0.5us of queue time)
                        for q2 in range(2):
                            nc.scalar.activation(
                                ysb[:, q2 * 256:(q2 + 1) * 256],
                                yps[:, q2 * 256:(q2 + 1) * 256], AF.Copy)
                            eng = nc.sync if q2 == 0 else nc.scalar
                            eng.dma_start(
                                y_d[t * 128:(t + 1) * 128,
                                    g * 512 + q2 * 256: g * 512 + (q2 + 1) * 256],
                                ysb[:, q2 * 256:(q2 + 1) * 256])
                    elif t == NT - 2:
                        nc.scalar.activation(ysb[:], yps[:], AF.Copy)
                        # split the tail stores across engines/queues
                        for q2 in range(2):
                            eng = nc.sync if q2 == 0 else nc.scalar
                            eng.dma_start(
                                y_d[t * 128:(t + 1) * 128,
                                    g * 512 + q2 * 256: g * 512 + (q2 + 1) * 256],
                                ysb[:, q2 * 256:(q2 + 1) * 256])
                    else:
                        nc.scalar.activation(ysb[:], yps[:], AF.Copy)
                        nc.sync.dma_start(
                            y_d[t * 128:(t + 1) * 128, g * 512:(g + 1) * 512],
                            ysb[:])

            LAG = 3   # tiles between attn and wo
            for t in range(2, NT):
                # 2-tile transpose lag so the DVE normalize chain of tile
                # t-2 is surely done and the PE never waits on it; the
                # transpose runs before attn so the attn matmuls give the
                # DVE chain of tile t-1 extra slack
                pe_transpose(t - 2)
                attn_tile(t)
                if deferred:
                    # one deferred q^T elu per iteration (DVE slack)
                    dst, qsb = deferred.pop(0)
                    elu1_half(dst, qsb[:])
                if t >= LAG:
                    wo_tile(t - LAG)
            # drain: a wo tile between the last two transposes hides the
            # DVE normalize latency of tile NT-1
            pe_transpose(NT - 2)
            wo_tile(NT - 3)
            pe_transpose(NT - 1)
            wo_tile(NT - 2)
            wo_tile(NT - 1)


def _get_program(has_bias):
    key = (has_bias, MM_DT)
    if key not in _PROGRAMS:
        nc = bacc.Bacc("TRN2", target_bir_lowering=False, debug=False,
                       num_devices=N_CORES)
        _emit(nc, has_bias, MM_DT)
        nc.compile()
        _PROGRAMS[key] = nc
    return _PROGRAMS[key]


def _to_mm_np(a):
    """Convert fp32 array to the numpy dtype matching MM_DT."""
    if MM_DT == BF16:
        import ml_dtypes
        return np.ascontiguousarray(a.astype(ml_dtypes.bfloat16))
    return np.ascontiguousarray(a)


def _pack_rhs(w):
    # W [out,in] -> W.T half-major rhs layout [128, 2*8*512]:
    # [p, h*4096 + c*512 + n] = W.T[c*128+p, h*512+n]
    return _to_mm_np(
        w.T.reshape(8, 128, 2, 512).transpose(1, 2, 0, 3).reshape(128, 8 * D))


def _pack_xt(xs):
    # xs [TOK, D] -> x^T group-major: [p, g*4096 + c*512 + u] = xs[g*512+u, c*128+p]
    ng = NT // 4
    return _to_mm_np(
        xs.T.reshape(8, 128, ng, 512).transpose(1, 2, 0, 3).reshape(128, NT * 1024))


def kernel(x, Wq, bq, Wk, bk, Wv, bv, Wo, bo):
    global LAST_RESULT
    x = np.asarray(x, dtype=np.float32)
    Wq, Wk, Wv, Wo = (np.asarray(w, dtype=np.float32) for w in (Wq, Wk, Wv, Wo))
    bq, bk, bv, bo = (np.asarray(b, dtype=np.float32) for b in (bq, bk, bv, bo))

    has_bias = bool(np.any(bq) or np.any(bk) or np.any(bv))
    nc = _get_program(has_bias)
    shared = {
        "wkt": _pack_rhs(Wk),
        "wvt": _pack_rhs(Wv),
        "wqt": _pack_rhs(Wq),
        "wot": _pack_rhs(Wo),
        "ident": _to_mm_np(np.eye(128, dtype=np.float32)),
        "biases": np.concatenate([bq, bk, bv, bo]).reshape(1, 4096),
    }
    in_maps = []
    for c in range(N_CORES):
        b = c // 2
        h = c % 2
        m = dict(shared)
        m["xst"] = _pack_xt(x[b, h * TOK:(h + 1) * TOK, :])
        in_maps.append(m)

    res = run_bass_kernel_spmd(nc, in_maps, list(range(N_CORES)), trace=TRACE)
    LAST_RESULT = res

    y = np.empty((B, S, D), dtype=np.float32)
    for c in range(N_CORES):
        b = c // 2
        h = c % 2
        y[b, h * TOK:(h + 1) * TOK, :] = np.asarray(
            res.results[c]["y"]).astype(np.float32)
    y += bo
    return y
